# revision 1
# baseline (speedup 1.0000x reference)
"""LoRA layer kernel for Trainium2: out = (W + A@B) @ x.

Shapes (fp32): W [4096,4096], A [4096,16], B [16,4096], x [4096,8192],
out [4096,8192].

Strategy (tensor-parallel, 8 NeuronCores):
- Shard W and A row-wise (output dim): 512 rows per core. Replicate B, x.
- Per core, on device:
    1. Load W_shard^T (fp32) into SBUF.
    2. delta^T = B^T @ A_shard^T via 32 K=16 fp32 matmuls (PSUM), add into
       W^T in fp32, round result to fp16 (W'^T).
    3. Stream x (pre-cast to fp16 on host) in 16 n-tiles of 512 columns;
       for each n-tile compute the 512x512 output block column via
       4 (m) x 32 (k) fp16 matmuls accumulating fp32 in PSUM; evict to
       SBUF fp32 and DMA to HBM.
- Host gathers the 8 row-shards into the full fp32 output.

fp16 is used for the big matmul because the PE runs fp16 at 1 cycle/row
(same as bf16, 4x faster than fp32) with fp32 PSUM accumulation; fp16's
10-bit mantissa keeps the scale-relative error at ~3e-4.
"""

import numpy as np

import concourse.bacc as bacc
import concourse.mybir as mybir
import concourse.tile as tile
from concourse.bass_utils import run_bass_kernel_spmd

P = 128          # partitions / systolic dim
OUT = 4096
IN = 4096
RANK = 16
NTOK = 8192
NCORES = 8
MSH = OUT // NCORES          # 512 output rows per core
KS = IN // P                 # 32 k-subtiles
MO = MSH // P                # 4 m-subtiles per core
NT = 16                      # n-tiles
NF = NTOK // NT              # 512 columns per n-tile


def _build_nc():
    nc = bacc.Bacc(None, target_bir_lowering=False, debug=False)

    wT = nc.dram_tensor("wT", [P, KS, MSH], mybir.dt.float32, kind="ExternalInput")
    aT = nc.dram_tensor("aT", [RANK, MSH], mybir.dt.float32, kind="ExternalInput")
    b = nc.dram_tensor("b", [RANK, KS, P], mybir.dt.float32, kind="ExternalInput")
    xh = nc.dram_tensor("xh", [NT, P, KS, NF], mybir.dt.float16, kind="ExternalInput")
    out = nc.dram_tensor("out", [NT, P, MO, NF], mybir.dt.float32, kind="ExternalOutput")

    with tile.TileContext(nc) as tc:
        with (
            tc.tile_pool(name="wpool", bufs=1) as wpool,
            tc.tile_pool(name="w16pool", bufs=1) as w16pool,
            tc.tile_pool(name="spool", bufs=1) as spool,
            tc.tile_pool(name="xpool", bufs=2) as xpool,
            tc.tile_pool(name="opool", bufs=2) as opool,
            tc.tile_pool(name="dpsum", bufs=2, space="PSUM") as dpsum,
            tc.tile_pool(name="mpsum", bufs=5, space="PSUM") as mpsum,
        ):
            wT_sb = wpool.tile([P, KS, MSH], mybir.dt.float32)
            wp16 = w16pool.tile([P, KS, MSH], mybir.dt.float16)
            aT_sb = spool.tile([RANK, MSH], mybir.dt.float32)
            b_sb = spool.tile([RANK, KS, P], mybir.dt.float32)

            nc.sync.dma_start(wT_sb[:], wT.ap())
            nc.sync.dma_start(aT_sb[:], aT.ap())
            nc.sync.dma_start(b_sb[:], b.ap())

            # W'^T = W^T + (A@B)^T, rounded to fp16.
            # Per ko: psum[p, m] = sum_r b[r, ko*128+p] * aT[r, m]  (K=16 matmul)
            for ko in range(KS):
                dps = dpsum.tile([P, MSH], mybir.dt.float32, tag="dps")
                nc.tensor.matmul(dps[:], b_sb[:, ko], aT_sb[:], start=True, stop=True)
                nc.vector.tensor_add(wT_sb[:, ko], wT_sb[:, ko], dps[:])
                nc.vector.tensor_copy(wp16[:, ko], wT_sb[:, ko])

            # Main: out[m, n] = sum_k W'[m, k] x[k, n], fp16 inputs, fp32 accum.
            for nt in range(NT):
                xt = xpool.tile([P, KS, NF], mybir.dt.float16, tag="xt")
                nc.sync.dma_start(xt[:], xh.ap()[nt])
                ot = opool.tile([P, MO, NF], mybir.dt.float32, tag="ot")
                for mo in range(MO):
                    mps = mpsum.tile([P, NF], mybir.dt.float32, tag="mps")
                    for ko in range(KS):
                        nc.tensor.matmul(
                            mps[:],
                            wp16[:, ko, mo * P:(mo + 1) * P],
                            xt[:, ko],
                            start=(ko == 0),
                            stop=(ko == KS - 1),
                        )
                    nc.vector.tensor_copy(ot[:, mo], mps[:])
                nc.sync.dma_start(out.ap()[nt], ot[:])

    nc.compile()
    return nc


_NC_CACHE = None


def _get_nc():
    global _NC_CACHE
    if _NC_CACHE is None:
        _NC_CACHE = _build_nc()
    return _NC_CACHE


def prepare_in_maps(x, weight, A, B):
    """Shard + lay out the full inputs into per-core device input maps."""
    x = np.ascontiguousarray(x, dtype=np.float32)
    weight = np.ascontiguousarray(weight, dtype=np.float32)
    A = np.ascontiguousarray(A, dtype=np.float32)
    B = np.ascontiguousarray(B, dtype=np.float32)

    # x [IN, NTOK] -> fp16 [nt, p, ko, j] with k = ko*128+p, n = nt*512+j
    xh = np.ascontiguousarray(
        x.astype(np.float16).reshape(KS, P, NT, NF).transpose(2, 1, 0, 3)
    )
    # B [RANK, IN] -> [r, ko, p]
    b_dev = np.ascontiguousarray(B.reshape(RANK, KS, P))

    in_maps = []
    for c in range(NCORES):
        rows = slice(c * MSH, (c + 1) * MSH)
        # W_shard^T [k, m] -> [p, ko, m]
        wT_dev = np.ascontiguousarray(
            weight[rows].T.reshape(KS, P, MSH).transpose(1, 0, 2)
        )
        aT_dev = np.ascontiguousarray(A[rows].T)
        in_maps.append({"wT": wT_dev, "aT": aT_dev, "b": b_dev, "xh": xh})
    return in_maps


def assemble_output(results):
    """Gather per-core [nt, p, mo, j] outputs into the full [OUT, NTOK] fp32."""
    out = np.empty((OUT, NTOK), dtype=np.float32)
    for c, r in enumerate(results):
        # [nt, p, mo, j] -> [mo, p, nt, j] -> [512, 8192]
        shard = r["out"].transpose(2, 1, 0, 3).reshape(MSH, NTOK)
        out[c * MSH:(c + 1) * MSH] = shard
    return out


def kernel(x, weight, A, B):
    nc = _get_nc()
    in_maps = prepare_in_maps(x, weight, A, B)
    res = run_bass_kernel_spmd(nc, in_maps, core_ids=list(range(NCORES)))
    return assemble_output(res.results)


if __name__ == "__main__":
    rng = np.random.default_rng(0)
    x = rng.standard_normal((IN, NTOK), dtype=np.float32)
    weight = rng.standard_normal((OUT, IN), dtype=np.float32)
    A = rng.standard_normal((OUT, RANK), dtype=np.float32)
    B = rng.standard_normal((RANK, IN), dtype=np.float32)
    got = kernel(x, weight, A, B)
    ref = (weight.astype(np.float64) + A.astype(np.float64) @ B.astype(np.float64)) @ x.astype(np.float64)
    err = np.abs(got - ref).max() / np.abs(ref).max()
    rel = np.linalg.norm(got - ref) / np.linalg.norm(ref)
    print("max-rel-to-max err:", err, " norm-rel:", rel)


# revision 2
# speedup vs baseline: 143.2389x; 143.2389x over previous
"""LoRA layer kernel for Trainium2: out = (W + A@B) @ x.

Shapes (fp32): W [4096,4096], A [4096,16], B [16,4096], x [4096,8192],
out [4096,8192].

Strategy (tensor-parallel, 8 NeuronCores):
- Shard W and A row-wise (output dim): 512 rows per core. Replicate B, x.
- Per core, on device:
    1. Load W_shard^T (fp32) into SBUF in 8 ko-group chunks (gpsimd DMA ring,
       so the x-load ring stays free).
    2. delta^T = B^T @ A_shard^T via 32 K=16 fp32 matmuls (PSUM); add into
       W^T in fp32 (VectorE), round result to fp16 (W'^T). These matmuls
       overlap the W DMA chunks via the shared 8-bank PSUM pool.
    3. Stream x (pre-cast to fp16 on host) in 16 n-tiles of 512 columns on
       the sync-engine DMA ring; for each n-tile compute the 512x512 output
       block via 4 (m) x 32 (k) fp16 matmuls accumulating fp32 in PSUM;
       evict to SBUF fp32 (VectorE) and DMA out on the gpsimd ring.
- Host gathers the 8 row-shards into the full fp32 output.

fp16 is used for the big matmul because the PE runs 16-bit matmuls at
1 column/cycle (4x faster than fp32) with fp32 PSUM accumulation; fp16's
10-bit mantissa keeps the scale-relative error at ~3e-4 (hardware-measured),
vs ~2e-3 for bf16.
"""

import numpy as np

import concourse.bacc as bacc
import concourse.mybir as mybir
import concourse.tile as tile
from concourse.bass_utils import run_bass_kernel_spmd

P = 128          # partitions / systolic dim
OUT = 4096
IN = 4096
RANK = 16
NTOK = 8192
NCORES = 8
MSH = OUT // NCORES          # 512 output rows per core
KS = IN // P                 # 32 k-subtiles
MO = MSH // P                # 4 m-subtiles per core
NT = 16                      # n-tiles
NF = NTOK // NT              # 512 columns per n-tile
NG = 8                       # W ko-group chunks
KG = KS // NG                # 4 ko per group


def _build_nc():
    nc = bacc.Bacc(None, target_bir_lowering=False, debug=False)

    wT = nc.dram_tensor("wT", [NG, P, KG, MSH], mybir.dt.float32, kind="ExternalInput")
    aT = nc.dram_tensor("aT", [RANK, MSH], mybir.dt.float32, kind="ExternalInput")
    b = nc.dram_tensor("b", [RANK, KS, P], mybir.dt.float32, kind="ExternalInput")
    xh = nc.dram_tensor("xh", [NT, P, KS, NF], mybir.dt.float16, kind="ExternalInput")
    out = nc.dram_tensor("out", [NT, P, MO, NF], mybir.dt.float32, kind="ExternalOutput")

    with tile.TileContext(nc) as tc:
        with (
            tc.tile_pool(name="wpool", bufs=1) as wpool,
            tc.tile_pool(name="w16pool", bufs=1) as w16pool,
            tc.tile_pool(name="spool", bufs=1) as spool,
            tc.tile_pool(name="xpool", bufs=2) as xpool,
            tc.tile_pool(name="opool", bufs=2) as opool,
            tc.tile_pool(name="psum", bufs=8, space="PSUM") as psum,
        ):
            aT_sb = spool.tile([RANK, MSH], mybir.dt.float32)
            b_sb = spool.tile([RANK, KS, P], mybir.dt.float32)
            nc.gpsimd.dma_start(aT_sb[:], aT.ap())
            nc.gpsimd.dma_start(b_sb[:], b.ap())

            wT_g, wp16_g = [], []
            for g in range(NG):
                wt = wpool.tile([P, KG, MSH], mybir.dt.float32,
                                tag=f"wt{g}", bufs=1, name=f"wt{g}")
                nc.gpsimd.dma_start(wt[:], wT.ap()[g])
                wT_g.append(wt)
                wp16_g.append(w16pool.tile([P, KG, MSH], mybir.dt.float16,
                                           tag=f"wp{g}", bufs=1, name=f"wp{g}"))

            # W'^T = W^T + (A@B)^T, rounded to fp16.
            # Per ko: psum[p, m] = sum_r b[r, ko*128+p] * aT[r, m]  (K=16 matmul)
            for ko in range(KS):
                g, kg = divmod(ko, KG)
                dps = psum.tile([P, MSH], mybir.dt.float32, tag="ps", name="dps")
                nc.tensor.matmul(dps[:], b_sb[:, ko], aT_sb[:], start=True, stop=True)
                nc.vector.tensor_add(wT_g[g][:, kg], wT_g[g][:, kg], dps[:])
                nc.vector.tensor_copy(wp16_g[g][:, kg], wT_g[g][:, kg])

            # Main: out[m, n] = sum_k W'[m, k] x[k, n], fp16 inputs, fp32 accum.
            for nt in range(NT):
                xt = xpool.tile([P, KS, NF], mybir.dt.float16, tag="xt", name="xt")
                nc.sync.dma_start(xt[:], xh.ap()[nt])
                ot = opool.tile([P, MO, NF], mybir.dt.float32, tag="ot", name="ot")
                for mo in range(MO):
                    mps = psum.tile([P, NF], mybir.dt.float32, tag="ps", name="mps")
                    for ko in range(KS):
                        g, kg = divmod(ko, KG)
                        nc.tensor.matmul(
                            mps[:],
                            wp16_g[g][:, kg, mo * P:(mo + 1) * P],
                            xt[:, ko],
                            start=(ko == 0),
                            stop=(ko == KS - 1),
                        )
                    nc.vector.tensor_copy(ot[:, mo], mps[:])
                nc.gpsimd.dma_start(out.ap()[nt], ot[:])

    nc.compile()
    return nc


_NC_CACHE = None


def _get_nc():
    global _NC_CACHE
    if _NC_CACHE is None:
        _NC_CACHE = _build_nc()
    return _NC_CACHE


def prepare_in_maps(x, weight, A, B):
    """Shard + lay out the full inputs into per-core device input maps."""
    x = np.ascontiguousarray(x, dtype=np.float32)
    weight = np.ascontiguousarray(weight, dtype=np.float32)
    A = np.ascontiguousarray(A, dtype=np.float32)
    B = np.ascontiguousarray(B, dtype=np.float32)

    # x [IN, NTOK] -> fp16 [nt, p, ko, j] with k = ko*128+p, n = nt*512+j
    xh = np.ascontiguousarray(
        x.astype(np.float16).reshape(KS, P, NT, NF).transpose(2, 1, 0, 3)
    )
    # B [RANK, IN] -> [r, ko, p]
    b_dev = np.ascontiguousarray(B.reshape(RANK, KS, P))

    in_maps = []
    for c in range(NCORES):
        rows = slice(c * MSH, (c + 1) * MSH)
        # W_shard^T [k, m] -> [g, p, kg, m] with k = (g*KG + kg)*P + p
        wT_dev = np.ascontiguousarray(
            weight[rows].T.reshape(NG, KG, P, MSH).transpose(0, 2, 1, 3)
        )
        aT_dev = np.ascontiguousarray(A[rows].T)
        in_maps.append({"wT": wT_dev, "aT": aT_dev, "b": b_dev, "xh": xh})
    return in_maps


def assemble_output(results):
    """Gather per-core [nt, p, mo, j] outputs into the full [OUT, NTOK] fp32."""
    out = np.empty((OUT, NTOK), dtype=np.float32)
    for c, r in enumerate(results):
        shard = r["out"].transpose(2, 1, 0, 3).reshape(MSH, NTOK)
        out[c * MSH:(c + 1) * MSH] = shard
    return out


def kernel(x, weight, A, B):
    nc = _get_nc()
    in_maps = prepare_in_maps(x, weight, A, B)
    res = run_bass_kernel_spmd(nc, in_maps, core_ids=list(range(NCORES)))
    return assemble_output(res.results)


if __name__ == "__main__":
    rng = np.random.default_rng(0)
    x = rng.standard_normal((IN, NTOK), dtype=np.float32)
    weight = rng.standard_normal((OUT, IN), dtype=np.float32)
    A = rng.standard_normal((OUT, RANK), dtype=np.float32)
    B = rng.standard_normal((RANK, IN), dtype=np.float32)
    got = kernel(x, weight, A, B)
    ref = (weight.astype(np.float64) + A.astype(np.float64) @ B.astype(np.float64)) @ x.astype(np.float64)
    err = np.abs(got - ref).max() / np.abs(ref).max()
    rel = np.linalg.norm(got - ref) / np.linalg.norm(ref)
    print("max-rel-to-max err:", err, " norm-rel:", rel)
